# revision 1
# baseline (speedup 1.0000x reference)
"""KNN grouped-vector-attention pool kernel for 8 Trainium2 NeuronCores.

Strategy: shard queries M=16384 across 8 cores (2048 each). Each core gathers
its 2048*16 = 32768 neighbor rows from a replicated combined table
[context_feat | context_coord | pad] (528B rows) via indirect DMA, then does
all projections on-chip in channel-transposed layout [C=128 partitions, rows].
"""
import sys
sys.path.insert(0, '/opt/trn_rl_repo')
import numpy as np

N_CORES = 8
M, N, K, C, G = 16384, 131072, 16, 128, 8
M_LOC = M // N_CORES          # 2048 queries per core
R_LOC = M_LOC * K             # 32768 gathered rows per core
CW = 132                      # combined row: 128 feat + 3 coord + 1 pad
CHUNK = 512                   # rows per compute chunk (one PSUM bank)
GROUP = 16 * CHUNK            # 8192 rows per stacked group
N_GROUPS = R_LOC // GROUP     # 4
N_CHUNK_BLK = CHUNK // 128    # 4 gather blocks per chunk
EPS_BN = 1e-5

_compiled = None


def _build():
    from concourse import bacc, bass, mybir
    import concourse.tile as tile

    f32 = mybir.dt.float32
    i32 = mybir.dt.int32
    AF = mybir.ActivationFunctionType
    OP = mybir.AluOpType

    nc = bacc.Bacc("TRN2", target_bir_lowering=False, debug=False,
                   num_devices=N_CORES)

    # ---- DRAM tensors -------------------------------------------------
    d = {}
    def inp(name, shape):
        d[name] = nc.dram_tensor(name, shape, f32, kind="ExternalInput").ap()
    inp("ctxcat", (N, CW))
    d["knn_t"] = nc.dram_tensor("knn_t", (128, R_LOC // 128), i32,
                                kind="ExternalInput").ap()
    inp("qfT", (C, M_LOC))
    inp("qcT", (3, M_LOC))
    inp("Wq", (C, C)); inp("Wk", (C, C)); inp("Wv", (C, C))
    inp("Wp1", (3, C)); inp("nWp1", (3, C)); inp("Wp2", (C, C))
    inp("Ww1s", (C, 16 * C)); inp("nWw1s", (C, 16 * C))
    inp("W2bd", (C, C)); inp("Sel", (C, 16 * C)); inp("ident", (C, C))
    for nm in ("sq", "bq", "sk", "bk", "bv", "sp1", "bp1", "sw1", "bw1"):
        inp(nm, (C, 1))
    out_d = nc.dram_tensor("out", (M_LOC, C), f32, kind="ExternalOutput").ap()

    from contextlib import ExitStack
    est = ExitStack()
    with tile.TileContext(nc) as tc, est:
        cpool = est.enter_context(tc.tile_pool(name="const", bufs=1))
        gpool = est.enter_context(tc.tile_pool(name="gath", bufs=1))
        vpool = est.enter_context(tc.tile_pool(name="valp", bufs=2))
        spool = est.enter_context(tc.tile_pool(name="work", bufs=2))
        wfpool = est.enter_context(tc.tile_pool(name="wfin", bufs=2))
        opool = est.enter_context(tc.tile_pool(name="outp", bufs=1))
        # psum pools, one bank each
        ps = {}
        for nm, nb in [("trf", 1), ("trc", 1), ("kp", 1), ("vp", 1),
                       ("pebp", 1), ("pebxp", 1), ("stk", 1), ("wrp", 1)]:
            ps[nm] = est.enter_context(tc.tile_pool(name=nm, bufs=nb, space="PSUM"))

        # ---- constants into SBUF -------------------------------------
        ct = {}
        for nm, sh in [("qfT", (C, M_LOC)), ("qcT", (3, M_LOC)),
                       ("Wq", (C, C)), ("Wk", (C, C)), ("Wv", (C, C)),
                       ("Wp1", (3, C)), ("nWp1", (3, C)), ("Wp2", (C, C)),
                       ("Ww1s", (C, 16 * C)), ("nWw1s", (C, 16 * C)),
                       ("W2bd", (C, C)), ("Sel", (C, 16 * C)),
                       ("ident", (C, C))]:
            ct[nm] = cpool.tile(list(sh), f32, tag=f"c_{nm}", name=f"c_{nm}")
            nc.sync.dma_start(out=ct[nm][:], in_=d[nm][:])
        for nm in ("sq", "bq", "sk", "bk", "bv", "sp1", "bp1", "sw1", "bw1"):
            ct[nm] = cpool.tile([C, 1], f32, tag=f"c_{nm}", name=f"c_{nm}")
            nc.sync.dma_start(out=ct[nm][:], in_=d[nm][:])
        knn_t = cpool.tile([128, R_LOC // 128], i32)
        nc.sync.dma_start(out=knn_t[:], in_=d["knn_t"][:])

        # ---- qT = relu(bn(Wq.T @ qfT)) [C, M_LOC] --------------------
        qT = cpool.tile([C, M_LOC], f32)
        for t in range(M_LOC // 512):
            q_ps = ps["kp"].tile([C, 512], f32, tag="kp_t", name="q_ps")
            nc.tensor.matmul(out=q_ps[:], lhsT=ct["Wq"][:],
                             rhs=ct["qfT"][:, t * 512:(t + 1) * 512],
                             start=True, stop=True)
            nc.scalar.activation(out=qT[:, t * 512:(t + 1) * 512], in_=q_ps[:],
                                 func=AF.Relu, bias=ct["bq"][:], scale=ct["sq"][:])

        outT = opool.tile([C, M_LOC], f32)

        for g in range(N_GROUPS):
            g_t = gpool.tile([128, (GROUP // 128) * CW], f32, tag="gath")
            valT = vpool.tile([C, GROUP], f32, tag="valp")
            stacked_ps = ps["stk"].tile([128, CHUNK], f32, tag="stk_t", name="stacked_ps")
            # -------- phase A: per chunk ------------------------------
            for i in range(16):
                ch = g * 16 + i              # global chunk id
                q0 = ch * 32                 # first query of chunk
                # gather 4 blocks of 128 rows
                for b in range(N_CHUNK_BLK):
                    blk = i * N_CHUNK_BLK + b      # block within group
                    gcol = ch * N_CHUNK_BLK + b    # global block = idx column
                    nc.gpsimd.indirect_dma_start(
                        out=g_t[:, blk * CW:(blk + 1) * CW],
                        out_offset=None,
                        in_=d["ctxcat"][:],
                        in_offset=bass.IndirectOffsetOnAxis(
                            ap=knn_t[:, gcol:gcol + 1], axis=0),
                    )
                # transpose feat blocks -> [C, 512]
                trf = ps["trf"].tile([128, CHUNK], f32, tag="trf_t", name="trf")
                trc = ps["trc"].tile([128, CHUNK], f32, tag="trc_t", name="trc")
                for b in range(N_CHUNK_BLK):
                    blk = i * N_CHUNK_BLK + b
                    nc.tensor.transpose(
                        out=trf[:, b * 128:(b + 1) * 128],
                        in_=g_t[:, blk * CW:blk * CW + 128],
                        identity=ct["ident"][:])
                    nc.tensor.transpose(
                        out=trc[0:3, b * 128:(b + 1) * 128],
                        in_=g_t[:, blk * CW + 128:blk * CW + 131],
                        identity=ct["ident"][:])
                ctxT = spool.tile([C, CHUNK], f32, tag="ctxT")
                nc.vector.tensor_copy(out=ctxT[:], in_=trf[:])
                ccT = spool.tile([3, CHUNK], f32, tag="ccT")
                nc.vector.tensor_copy(out=ccT[:], in_=trc[0:3, :])
                # k / v projections
                k_ps = ps["kp"].tile([C, CHUNK], f32, tag="kp_t", name="k_ps")
                nc.tensor.matmul(out=k_ps[:], lhsT=ct["Wk"][:], rhs=ctxT[:],
                                 start=True, stop=True)
                keyT = spool.tile([C, CHUNK], f32, tag="keyT")
                nc.scalar.activation(out=keyT[:], in_=k_ps[:], func=AF.Relu,
                                     bias=ct["bk"][:], scale=ct["sk"][:])
                # pebx = relu(bn(Wp1.T @ (ccT - qc_bcast)))
                pebx_ps = ps["pebxp"].tile([C, CHUNK], f32, tag="pebxp_t", name="pebx_ps")
                qc_rep = ct["qcT"][:, q0:q0 + 32].unsqueeze(2) \
                    .to_broadcast([3, 32, K])
                nc.tensor.matmul(out=pebx_ps[:], lhsT=ct["Wp1"][:], rhs=ccT[:],
                                 start=True, stop=False)
                nc.tensor.matmul(out=pebx_ps[:], lhsT=ct["nWp1"][:], rhs=qc_rep,
                                 start=False, stop=True)
                pebxT = spool.tile([C, CHUNK], f32, tag="pebxT")
                nc.scalar.activation(out=pebxT[:], in_=pebx_ps[:], func=AF.Relu,
                                     bias=ct["bp1"][:], scale=ct["sp1"][:])
                # peb (twice: own bank + accumulated into v bank)
                peb_ps = ps["pebp"].tile([C, CHUNK], f32, tag="pebp_t", name="peb_ps")
                nc.tensor.matmul(out=peb_ps[:], lhsT=ct["Wp2"][:], rhs=pebxT[:],
                                 start=True, stop=True)
                v_ps = ps["vp"].tile([C, CHUNK], f32, tag="vp_t", name="v_ps")
                nc.tensor.matmul(out=v_ps[:], lhsT=ct["Wv"][:], rhs=ctxT[:],
                                 start=True, stop=False)
                nc.tensor.matmul(out=v_ps[:], lhsT=ct["Wp2"][:], rhs=pebxT[:],
                                 start=False, stop=True)
                nc.scalar.activation(out=valT[:, i * CHUNK:(i + 1) * CHUNK],
                                     in_=v_ps[:], func=AF.Identity,
                                     bias=ct["bv"][:], scale=1.0)
                # rel' = keyT + peb  (q folded into wl via nWw1s)
                relT = spool.tile([C, CHUNK], f32, tag="relT")
                nc.vector.tensor_tensor(out=relT[:], in0=keyT[:], in1=peb_ps[:],
                                        op=OP.add)
                # wl stripes into stacked psum
                q_rep = qT[:, q0:q0 + 32].unsqueeze(2).to_broadcast([C, 32, K])
                nc.tensor.matmul(out=stacked_ps[:],
                                 lhsT=ct["Ww1s"][:, i * C:(i + 1) * C],
                                 rhs=relT[:], start=(i == 0), stop=False,
                                 skip_group_check=True)
                nc.tensor.matmul(out=stacked_ps[:],
                                 lhsT=ct["nWw1s"][:, i * C:(i + 1) * C],
                                 rhs=q_rep, start=False, stop=(i == 15),
                                 skip_group_check=True)
            # -------- group tail: bn/relu, mm2, softmax ---------------
            stk_bn = spool.tile([128, CHUNK], f32, tag="stkbn")
            nc.scalar.activation(out=stk_bn[:], in_=stacked_ps[:], func=AF.Relu,
                                 bias=ct["bw1"][:], scale=ct["sw1"][:])
            w2_ps = ps["trf"].tile([128, CHUNK], f32, tag="trf_t", name="w2_ps")
            nc.tensor.matmul(out=w2_ps[:], lhsT=ct["W2bd"][:], rhs=stk_bn[:],
                             start=True, stop=True)
            mx = spool.tile([128, 32], f32, tag="mx")
            nc.vector.tensor_reduce(
                out=mx[:], in_=w2_ps[:].rearrange("p (m k) -> p m k", k=K),
                axis=mybir.AxisListType.X, op=OP.max)
            sm = spool.tile([128, CHUNK], f32, tag="sm")
            nc.vector.tensor_tensor(
                out=sm[:].rearrange("p (m k) -> p m k", k=K),
                in0=w2_ps[:].rearrange("p (m k) -> p m k", k=K),
                in1=mx[:].unsqueeze(2).to_broadcast([128, 32, K]),
                op=OP.subtract)
            e_t = spool.tile([128, CHUNK], f32, tag="e")
            nc.scalar.activation(out=e_t[:], in_=sm[:], func=AF.Exp)
            s_t = spool.tile([128, 32], f32, tag="s")
            nc.vector.tensor_reduce(
                out=s_t[:], in_=e_t[:].rearrange("p (m k) -> p m k", k=K),
                axis=mybir.AxisListType.X, op=OP.add)
            rinv = spool.tile([128, 32], f32, tag="rinv")
            nc.vector.reciprocal(out=rinv[:], in_=s_t[:])
            wfin = wfpool.tile([128, CHUNK], f32, tag="wfin")
            nc.vector.tensor_tensor(
                out=wfin[:].rearrange("p (m k) -> p m k", k=K),
                in0=e_t[:].rearrange("p (m k) -> p m k", k=K),
                in1=rinv[:].unsqueeze(2).to_broadcast([128, 32, K]),
                op=OP.mult)
            # -------- phase B: weighted sum per chunk -----------------
            for i in range(16):
                ch = g * 16 + i
                wrep_ps = ps["wrp"].tile([C, CHUNK], f32, tag="wrp_t", name="wrep_ps")
                nc.tensor.matmul(out=wrep_ps[:],
                                 lhsT=ct["Sel"][:, i * C:(i + 1) * C],
                                 rhs=wfin[:], start=True, stop=True)
                prod = spool.tile([C, CHUNK], f32, tag="prod")
                nc.vector.tensor_tensor(out=prod[:],
                                        in0=valT[:, i * CHUNK:(i + 1) * CHUNK],
                                        in1=wrep_ps[:], op=OP.mult)
                nc.vector.tensor_reduce(
                    out=outT[:, ch * 32:(ch + 1) * 32],
                    in_=prod[:].rearrange("p (m k) -> p m k", k=K),
                    axis=mybir.AxisListType.X, op=OP.add)

        # ---- transpose outT -> out [M_LOC, C] and store --------------
        for t in range(M_LOC // 128):
            o_ps = ps["trc"].tile([128, CHUNK], f32, tag="trc_t", name="o_ps")
            nc.tensor.transpose(out=o_ps[:, 0:128], in_=outT[:, t * 128:(t + 1) * 128],
                                identity=ct["ident"][:])
            o_sb = spool.tile([128, 128], f32, tag="osb")
            nc.vector.tensor_copy(out=o_sb[:], in_=o_ps[:, 0:128])
            nc.sync.dma_start(out=out_d[t * 128:(t + 1) * 128, :], in_=o_sb[:])

    nc.compile()
    return nc


def _prep_inputs(inputs):
    """Host-side marshaling: shard queries, build combined table + consts."""
    f = np.float32
    ctx_f = np.asarray(inputs["context_feat"], f)
    ctx_c = np.asarray(inputs["context_coord"], f)
    ctxcat = np.zeros((N, CW), f)
    ctxcat[:, :C] = ctx_f
    ctxcat[:, C:C + 3] = ctx_c

    s = lambda g: (np.asarray(g, f) / np.sqrt(np.float32(1.0 + EPS_BN)))
    Wq = np.asarray(inputs["Wq"], f); Wk = np.asarray(inputs["Wk"], f)
    Wv = np.asarray(inputs["Wv"], f)
    Wp1 = np.asarray(inputs["Wp1"], f); Wp2 = np.asarray(inputs["Wp2"], f)
    Ww1 = np.asarray(inputs["Ww1"], f); Ww2 = np.asarray(inputs["Ww2"], f)

    sq = s(inputs["gq"]); bq = sq * inputs["bq"] + np.asarray(inputs["betaq"], f)
    sk = s(inputs["gk"]); bk = sk * inputs["bk"] + np.asarray(inputs["betak"], f)
    sp1 = s(inputs["gp1"])
    bp1 = sp1 * inputs["bp1"] + np.asarray(inputs["betap1"], f)
    bv = np.asarray(inputs["bv"], f) + np.asarray(inputs["bp2"], f)  # val bias
    # stacked bn for w1: row 8i+g ; fold bp2@Ww1 into bias
    sw1_g = s(inputs["gw1"])                                   # [G]
    bw1_g = (sw1_g * (np.asarray(inputs["bw1"], f)
                      + np.asarray(inputs["bp2"], f) @ Ww1)
             + np.asarray(inputs["betaw1"], f))                # [G]
    sw1 = np.tile(sw1_g, 16).astype(f)
    bw1 = np.tile(bw1_g, 16).astype(f)

    Ww1s = np.zeros((C, 16 * C), f)
    Sel = np.zeros((C, 16 * C), f)
    W2bd = np.zeros((C, C), f)
    for i in range(16):
        Ww1s[:, i * C + 8 * i: i * C + 8 * i + 8] = Ww1
        blockc = np.zeros((C, C), f)
        blockc[8 * i + np.arange(C) // 16, np.arange(C)] = 1.0
        Sel[:, i * C:(i + 1) * C] = blockc
    for i in range(16):
        W2bd[8 * i:8 * i + 8, 8 * i:8 * i + 8] = Ww2

    col = lambda v: np.asarray(v, f).reshape(C, 1)
    base = {
        "ctxcat": ctxcat, "Wq": Wq, "Wk": Wk, "Wv": Wv,
        "Wp1": Wp1, "nWp1": -Wp1, "Wp2": Wp2,
        "Ww1s": Ww1s, "nWw1s": -Ww1s, "W2bd": W2bd, "Sel": Sel,
        "ident": np.eye(C, dtype=f),
        "sq": col(sq), "bq": col(bq), "sk": col(sk), "bk": col(bk),
        "bv": col(bv), "sp1": np.zeros((C, 1), f), "bp1": np.zeros((C, 1), f),
        "sw1": col(sw1), "bw1": col(bw1),
    }
    base["sp1"][:, 0] = sp1
    base["bp1"][:, 0] = bp1

    knn = np.asarray(inputs["knn_indexes"])
    knn = np.where(knn < 0, 0, knn).astype(np.int32)
    qf = np.asarray(inputs["query_feat"], f)
    qc = np.asarray(inputs["query_coord"], f)

    in_maps = []
    for c in range(N_CORES):
        sl = slice(c * M_LOC, (c + 1) * M_LOC)
        flat = knn[sl].reshape(-1)                       # [R_LOC] m*16+k order
        knn_t = flat.reshape(R_LOC // 128, 128).T.copy() # [128, R_LOC/128]
        m = dict(base)
        m["knn_t"] = knn_t
        m["qfT"] = qf[sl].T.copy()
        m["qcT"] = qc[sl].T.copy()
        in_maps.append(m)
    return in_maps


def kernel(**inputs):
    global _compiled
    from concourse.bass_utils import run_bass_kernel_spmd
    if _compiled is None:
        _compiled = _build()
    in_maps = _prep_inputs(inputs)
    res = run_bass_kernel_spmd(_compiled, in_maps, core_ids=list(range(N_CORES)))
    out = np.concatenate([res.results[c]["out"] for c in range(N_CORES)], axis=0)
    return out.astype(np.float32)


if __name__ == "__main__":
    import reference
    inputs = {k: np.asarray(v) for k, v in reference.setup_inputs().items()}
    got = kernel(**inputs)
    exp = np.asarray(reference.reference(**reference.setup_inputs()))
    err = np.abs(got - exp).max() / (np.abs(exp).max() + 1e-9)
    print("Relative error:", err)



# revision 2
# speedup vs baseline: 4.9378x; 4.9378x over previous
"""KNN grouped-vector-attention pool kernel for 8 Trainium2 NeuronCores.

Strategy: shard queries M=16384 across 8 cores (2048 each). The KNN gather is
resolved on the host during input marshaling: each core receives its
2048*16 = 32768 neighbor rows already gathered, channel-transposed and cast to
fp16 (featT [128, R_LOC], relative positions posT [3, R_LOC]). This removes
the replicated 69MB context table per core (the dominant transfer cost) and
all on-device transposes. All projections run on-chip in channel-major layout;
fp16 is used for every matmul operand with fp32 PSUM accumulation.
"""
import sys
sys.path.insert(0, '/opt/trn_rl_repo')
import numpy as np

N_CORES = 8
M, N, K, C, G = 16384, 131072, 16, 128, 8
M_LOC = M // N_CORES          # 2048 queries per core
R_LOC = M_LOC * K             # 32768 gathered rows per core
CHUNK = 512                   # rows per compute chunk (one PSUM bank)
GROUP = 16 * CHUNK            # 8192 rows per stacked group
N_GROUPS = R_LOC // GROUP     # 4
EPS_BN = 1e-5

_compiled = None


def _build():
    from concourse import bacc, bass, mybir
    import concourse.tile as tile

    f32 = mybir.dt.float32
    f16 = mybir.dt.float16
    AF = mybir.ActivationFunctionType
    OP = mybir.AluOpType

    nc = bacc.Bacc("TRN2", target_bir_lowering=False, debug=False,
                   num_devices=N_CORES)

    # ---- DRAM tensors -------------------------------------------------
    d = {}
    def inp(name, shape, dt=f16):
        d[name] = nc.dram_tensor(name, shape, dt, kind="ExternalInput").ap()
    inp("featT", (C, R_LOC))
    inp("posT", (3, R_LOC))
    inp("qfT", (C, M_LOC))
    inp("Wq", (C, C)); inp("Wk", (C, C)); inp("Wv", (C, C))
    inp("Wp1", (3, C)); inp("Wp2", (C, C))
    inp("Ww1s", (C, 16 * C)); inp("WpW1s", (C, 16 * C))
    inp("W2bd", (C, C)); inp("Sel", (C, 16 * C))
    for nm in ("sq", "bq", "sk", "bk", "bv", "sp1", "bp1", "sw1", "bw1"):
        inp(nm, (C, 1), f32)
    out_d = nc.dram_tensor("out", (C, M_LOC), f32, kind="ExternalOutput").ap()

    from contextlib import ExitStack
    est = ExitStack()
    with tile.TileContext(nc) as tc, est:
        cpool = est.enter_context(tc.tile_pool(name="const", bufs=1))
        gpool = est.enter_context(tc.tile_pool(name="gath", bufs=2))
        vpool = est.enter_context(tc.tile_pool(name="valp", bufs=2))
        spool = est.enter_context(tc.tile_pool(name="work", bufs=2))
        opool = est.enter_context(tc.tile_pool(name="outp", bufs=1))
        ps = {}
        for nm, nb in [("kp", 2), ("px", 2), ("vp", 2), ("stk", 1)]:
            ps[nm] = est.enter_context(tc.tile_pool(name=nm, bufs=nb, space="PSUM"))

        # ---- constants into SBUF -------------------------------------
        ct = {}
        for nm, sh in [("qfT", (C, M_LOC)),
                       ("Wq", (C, C)), ("Wk", (C, C)), ("Wv", (C, C)),
                       ("Wp1", (3, C)), ("Wp2", (C, C)),
                       ("Ww1s", (C, 16 * C)), ("WpW1s", (C, 16 * C)),
                       ("W2bd", (C, C)), ("Sel", (C, 16 * C))]:
            ct[nm] = cpool.tile(list(sh), f16, tag=f"c_{nm}", name=f"c_{nm}")
            nc.sync.dma_start(out=ct[nm][:], in_=d[nm][:])
        for nm in ("sq", "bq", "sk", "bk", "bv", "sp1", "bp1", "sw1", "bw1"):
            ct[nm] = cpool.tile([C, 1], f32, tag=f"c_{nm}", name=f"c_{nm}")
            nc.sync.dma_start(out=ct[nm][:], in_=d[nm][:])

        # ---- qT = relu(bn(Wq.T @ qfT)) fp16 [C, M_LOC]; nqT = -qT -----
        qT = cpool.tile([C, M_LOC], f16, tag="c_qT", name="c_qT")
        nqT = cpool.tile([C, M_LOC], f16, tag="c_nqT", name="c_nqT")
        for t in range(M_LOC // CHUNK):
            q_ps = ps["kp"].tile([C, CHUNK], f32, tag="kp_t", name="q_ps")
            nc.tensor.matmul(out=q_ps[:], lhsT=ct["Wq"][:],
                             rhs=ct["qfT"][:, t * CHUNK:(t + 1) * CHUNK],
                             start=True, stop=True)
            nc.scalar.activation(out=qT[:, t * CHUNK:(t + 1) * CHUNK],
                                 in_=q_ps[:], func=AF.Relu,
                                 bias=ct["bq"][:], scale=ct["sq"][:])
            nc.scalar.activation(out=nqT[:, t * CHUNK:(t + 1) * CHUNK],
                                 in_=qT[:, t * CHUNK:(t + 1) * CHUNK],
                                 func=AF.Identity, scale=-1.0)

        outT = opool.tile([C, M_LOC], f32)

        for g in range(N_GROUPS):
            fT = gpool.tile([C, GROUP], f16, tag="fT")
            nc.sync.dma_start(out=fT[:], in_=d["featT"][:, g * GROUP:(g + 1) * GROUP])
            pT = gpool.tile([3, GROUP], f16, tag="pT")
            nc.sync.dma_start(out=pT[:], in_=d["posT"][:, g * GROUP:(g + 1) * GROUP])
            valT = vpool.tile([C, GROUP], f32, tag="valp")
            stacked_ps = ps["stk"].tile([C, CHUNK], f32, tag="stk_t", name="stacked_ps")
            # -------- phase A: per chunk of 512 gathered rows ---------
            for i in range(16):
                ch = g * 16 + i              # global chunk id
                q0 = ch * 32                 # first query of chunk
                ctx = fT[:, i * CHUNK:(i + 1) * CHUNK]
                pos = pT[:, i * CHUNK:(i + 1) * CHUNK]
                # key = relu(bn(Wk.T @ ctx))
                k_ps = ps["kp"].tile([C, CHUNK], f32, tag="kp_t", name="k_ps")
                nc.tensor.matmul(out=k_ps[:], lhsT=ct["Wk"][:], rhs=ctx,
                                 start=True, stop=True)
                keyT = spool.tile([C, CHUNK], f16, tag="keyT")
                nc.scalar.activation(out=keyT[:], in_=k_ps[:], func=AF.Relu,
                                     bias=ct["bk"][:], scale=ct["sk"][:])
                # pebx = relu(bn(Wp1.T @ pos))
                pebx_ps = ps["px"].tile([C, CHUNK], f32, tag="px_t", name="pebx_ps")
                nc.tensor.matmul(out=pebx_ps[:], lhsT=ct["Wp1"][:], rhs=pos,
                                 start=True, stop=True)
                pebxT = spool.tile([C, CHUNK], f16, tag="pebxT")
                nc.scalar.activation(out=pebxT[:], in_=pebx_ps[:], func=AF.Relu,
                                     bias=ct["bp1"][:], scale=ct["sp1"][:])
                # val = Wv.T @ ctx + Wp2.T @ pebx (+ bv + bp2 via bias)
                v_ps = ps["vp"].tile([C, CHUNK], f32, tag="vp_t", name="v_ps")
                nc.tensor.matmul(out=v_ps[:], lhsT=ct["Wv"][:], rhs=ctx,
                                 start=True, stop=False)
                nc.tensor.matmul(out=v_ps[:], lhsT=ct["Wp2"][:], rhs=pebxT[:],
                                 start=False, stop=True)
                nc.scalar.activation(out=valT[:, i * CHUNK:(i + 1) * CHUNK],
                                     in_=v_ps[:], func=AF.Identity,
                                     bias=ct["bv"][:], scale=1.0)
                # w1 logits, stacked: Ww1.T @ (key - q + peb) with
                # peb folded via WpW1s = Wp2 @ Ww1s and -q via nqT
                q_rep = nqT[:, q0:q0 + 32].unsqueeze(2).to_broadcast([C, 32, K])
                nc.tensor.matmul(out=stacked_ps[:],
                                 lhsT=ct["Ww1s"][:, i * C:(i + 1) * C],
                                 rhs=keyT[:], start=(i == 0), stop=False,
                                 skip_group_check=True)
                nc.tensor.matmul(out=stacked_ps[:],
                                 lhsT=ct["WpW1s"][:, i * C:(i + 1) * C],
                                 rhs=pebxT[:], start=False, stop=False,
                                 skip_group_check=True)
                nc.tensor.matmul(out=stacked_ps[:],
                                 lhsT=ct["Ww1s"][:, i * C:(i + 1) * C],
                                 rhs=q_rep, start=False, stop=(i == 15),
                                 skip_group_check=True)
            # -------- group tail: bn/relu, mm2, softmax ---------------
            stk_bn = spool.tile([C, CHUNK], f16, tag="stkbn")
            nc.scalar.activation(out=stk_bn[:], in_=stacked_ps[:], func=AF.Relu,
                                 bias=ct["bw1"][:], scale=ct["sw1"][:])
            w2_ps = ps["px"].tile([C, CHUNK], f32, tag="px_t", name="w2_ps")
            nc.tensor.matmul(out=w2_ps[:], lhsT=ct["W2bd"][:], rhs=stk_bn[:],
                             start=True, stop=True)
            mx = spool.tile([C, 32], f32, tag="mx")
            nc.vector.tensor_reduce(
                out=mx[:], in_=w2_ps[:].rearrange("p (m k) -> p m k", k=K),
                axis=mybir.AxisListType.X, op=OP.max)
            sm = spool.tile([C, CHUNK], f32, tag="sm")
            nc.vector.tensor_tensor(
                out=sm[:].rearrange("p (m k) -> p m k", k=K),
                in0=w2_ps[:].rearrange("p (m k) -> p m k", k=K),
                in1=mx[:].unsqueeze(2).to_broadcast([C, 32, K]),
                op=OP.subtract)
            e_t = spool.tile([C, CHUNK], f32, tag="e")
            nc.scalar.activation(out=e_t[:], in_=sm[:], func=AF.Exp)
            s_t = spool.tile([C, 32], f32, tag="s")
            nc.vector.tensor_reduce(
                out=s_t[:], in_=e_t[:].rearrange("p (m k) -> p m k", k=K),
                axis=mybir.AxisListType.X, op=OP.add)
            rinv = spool.tile([C, 32], f32, tag="rinv")
            nc.vector.reciprocal(out=rinv[:], in_=s_t[:])
            wf32 = spool.tile([C, CHUNK], f32, tag="wf32")
            nc.vector.tensor_tensor(
                out=wf32[:].rearrange("p (m k) -> p m k", k=K),
                in0=e_t[:].rearrange("p (m k) -> p m k", k=K),
                in1=rinv[:].unsqueeze(2).to_broadcast([C, 32, K]),
                op=OP.mult)
            wfin = spool.tile([C, CHUNK], f16, tag="wfin")
            nc.scalar.activation(out=wfin[:], in_=wf32[:], func=AF.Identity)
            # -------- phase B: weighted sum per chunk -----------------
            for i in range(16):
                ch = g * 16 + i
                wrep_ps = ps["kp"].tile([C, CHUNK], f32, tag="kp_t", name="wrep_ps")
                nc.tensor.matmul(out=wrep_ps[:],
                                 lhsT=ct["Sel"][:, i * C:(i + 1) * C],
                                 rhs=wfin[:], start=True, stop=True)
                prod = spool.tile([C, CHUNK], f32, tag="prod")
                nc.vector.tensor_tensor(out=prod[:],
                                        in0=valT[:, i * CHUNK:(i + 1) * CHUNK],
                                        in1=wrep_ps[:], op=OP.mult)
                nc.vector.tensor_reduce(
                    out=outT[:, ch * 32:(ch + 1) * 32],
                    in_=prod[:].rearrange("p (m k) -> p m k", k=K),
                    axis=mybir.AxisListType.X, op=OP.add)

        nc.sync.dma_start(out=out_d[:], in_=outT[:])

    nc.compile()
    return nc


def _prep_inputs(inputs):
    """Host-side marshaling: per-core KNN gather, transpose, fp16 cast."""
    f = np.float32
    h = np.float16
    ctx_f = np.asarray(inputs["context_feat"], f)
    ctx_c = np.asarray(inputs["context_coord"], f)

    s = lambda g_: (np.asarray(g_, f) / np.sqrt(np.float32(1.0 + EPS_BN)))
    Wq = np.asarray(inputs["Wq"], f); Wk = np.asarray(inputs["Wk"], f)
    Wv = np.asarray(inputs["Wv"], f)
    Wp1 = np.asarray(inputs["Wp1"], f); Wp2 = np.asarray(inputs["Wp2"], f)
    Ww1 = np.asarray(inputs["Ww1"], f); Ww2 = np.asarray(inputs["Ww2"], f)

    sq = s(inputs["gq"]); bq = sq * inputs["bq"] + np.asarray(inputs["betaq"], f)
    sk = s(inputs["gk"]); bk = sk * inputs["bk"] + np.asarray(inputs["betak"], f)
    sp1 = s(inputs["gp1"])
    bp1 = sp1 * inputs["bp1"] + np.asarray(inputs["betap1"], f)
    bv = np.asarray(inputs["bv"], f) + np.asarray(inputs["bp2"], f)  # val bias
    # stacked bn for w1: row 8i+g ; fold bp2@Ww1 into bias
    sw1_g = s(inputs["gw1"])                                   # [G]
    bw1_g = (sw1_g * (np.asarray(inputs["bw1"], f)
                      + np.asarray(inputs["bp2"], f) @ Ww1)
             + np.asarray(inputs["betaw1"], f))                # [G]
    sw1 = np.tile(sw1_g, 16).astype(f)
    bw1 = np.tile(bw1_g, 16).astype(f)

    P2W1 = (Wp2 @ Ww1).astype(f)                               # [C, G]
    Ww1s = np.zeros((C, 16 * C), f)
    WpW1s = np.zeros((C, 16 * C), f)
    Sel = np.zeros((C, 16 * C), f)
    W2bd = np.zeros((C, C), f)
    for i in range(16):
        Ww1s[:, i * C + 8 * i: i * C + 8 * i + 8] = Ww1
        WpW1s[:, i * C + 8 * i: i * C + 8 * i + 8] = P2W1
        blockc = np.zeros((C, C), f)
        blockc[8 * i + np.arange(C) // 16, np.arange(C)] = 1.0
        Sel[:, i * C:(i + 1) * C] = blockc
        W2bd[8 * i:8 * i + 8, 8 * i:8 * i + 8] = Ww2

    col = lambda v: np.ascontiguousarray(np.asarray(v, f).reshape(C, 1))
    base = {
        "Wq": Wq.astype(h), "Wk": Wk.astype(h), "Wv": Wv.astype(h),
        "Wp1": Wp1.astype(h), "Wp2": Wp2.astype(h),
        "Ww1s": Ww1s.astype(h), "WpW1s": WpW1s.astype(h),
        "W2bd": W2bd.astype(h), "Sel": Sel.astype(h),
        "sq": col(sq), "bq": col(bq), "sk": col(sk), "bk": col(bk),
        "bv": col(bv), "sp1": col(sp1), "bp1": col(bp1),
        "sw1": col(sw1), "bw1": col(bw1),
    }

    knn = np.asarray(inputs["knn_indexes"])
    knn = np.where(knn < 0, 0, knn).astype(np.int32)
    qf = np.asarray(inputs["query_feat"], f)
    qc = np.asarray(inputs["query_coord"], f)

    in_maps = []
    for c in range(N_CORES):
        sl = slice(c * M_LOC, (c + 1) * M_LOC)
        idx = knn[sl].reshape(-1)                        # [R_LOC] m*16+k order
        m = dict(base)
        m["featT"] = ctx_f[idx].T.astype(h)              # [C, R_LOC]
        pos = ctx_c[idx] - np.repeat(qc[sl], K, axis=0)  # [R_LOC, 3]
        m["posT"] = pos.T.astype(h)                      # [3, R_LOC]
        m["qfT"] = qf[sl].T.astype(h)
        in_maps.append(m)
    return in_maps


def kernel(**inputs):
    global _compiled
    from concourse.bass_utils import run_bass_kernel_spmd
    if _compiled is None:
        _compiled = _build()
    in_maps = _prep_inputs(inputs)
    res = run_bass_kernel_spmd(_compiled, in_maps, core_ids=list(range(N_CORES)))
    out = np.concatenate([res.results[c]["out"].T for c in range(N_CORES)], axis=0)
    return np.ascontiguousarray(out.astype(np.float32))


# revision 4
# speedup vs baseline: 8.5087x; 1.7232x over previous
"""KNN grouped-vector-attention pool kernel for 8 Trainium2 NeuronCores.

Strategy: shard queries M=16384 across 8 cores (2048 each). The KNN gather is
resolved on the host during input marshaling: each core receives its
2048*16 = 32768 neighbor rows already gathered, channel-transposed and cast to
fp16. All per-core inputs are packed into a single contiguous fp16 blob
(~9.3MB) so the host->device path pays one transfer per core instead of ~20
(per-array fixed cost dominates on this interconnect). Structured matrices
(Sel / Ww1s / WpW1s) are synthesized on device from tiny seeds. All matmuls
run fp16 x fp16 with fp32 PSUM accumulation; output returns as fp16.
"""
import sys
sys.path.insert(0, '/opt/trn_rl_repo')
import numpy as np

N_CORES = 8
M, N, K, C, G = 16384, 131072, 16, 128, 8
M_LOC = M // N_CORES          # 2048 queries per core
R_LOC = M_LOC * K             # 32768 gathered rows per core
CHUNK = 512                   # rows per compute chunk (one PSUM bank)
GROUP = 16 * CHUNK            # 8192 rows per stacked group
N_GROUPS = R_LOC // GROUP     # 4
EPS_BN = 1e-5

# blob layout: (name, (partitions, cols)) packed row-major, fp16
_LAYOUT = [
    ("featT", (C, R_LOC)),
    ("qfT", (C, M_LOC)),
    ("Wq", (C, C)), ("Wk", (C, C)), ("Wv", (C, C)),
    ("Wp2", (C, C)), ("W2bd", (C, C)),
    ("Ww1", (C, G)), ("P2W1", (C, G)),
    ("scal", (C, 9)),            # sq,bq,sk,bk,bv,sp1,bp1,sw1,bw1
    ("posT", (3, R_LOC)),
    ("Wp1", (3, C)),
]
_OFFS = {}
_NTOT = 0
for _nm, (_p, _c) in _LAYOUT:
    _OFFS[_nm] = _NTOT
    _NTOT += _p * _c

_compiled = None


def _build():
    from concourse import bacc, bass, mybir
    import concourse.tile as tile

    f32 = mybir.dt.float32
    f16 = mybir.dt.float16
    AF = mybir.ActivationFunctionType
    OP = mybir.AluOpType

    nc = bacc.Bacc("TRN2", target_bir_lowering=False, debug=False,
                   num_devices=N_CORES)

    blob = nc.dram_tensor("blob", (_NTOT,), f16, kind="ExternalInput").ap()
    out_d = nc.dram_tensor("out", (C, M_LOC), f16, kind="ExternalOutput").ap()

    def view(nm):
        p, c = dict(_LAYOUT)[nm]
        off = _OFFS[nm]
        return blob[off:off + p * c].rearrange("(p c) -> p c", p=p)

    from contextlib import ExitStack
    est = ExitStack()
    with tile.TileContext(nc) as tc, est:
        cpool = est.enter_context(tc.tile_pool(name="const", bufs=1))
        gpool = est.enter_context(tc.tile_pool(name="gath", bufs=2))
        vpool = est.enter_context(tc.tile_pool(name="valp", bufs=2))
        spool = est.enter_context(tc.tile_pool(name="work", bufs=2))
        opool = est.enter_context(tc.tile_pool(name="outp", bufs=1))
        ps = {}
        for nm, nb in [("kp", 2), ("px", 2), ("vp", 2), ("stk", 1)]:
            ps[nm] = est.enter_context(tc.tile_pool(name=nm, bufs=nb, space="PSUM"))

        # ---- constants into SBUF -------------------------------------
        ct = {}
        for nm in ("qfT", "Wq", "Wk", "Wv", "Wp2", "W2bd", "Ww1", "P2W1",
                   "scal", "Wp1"):
            p, c = dict(_LAYOUT)[nm]
            ct[nm] = cpool.tile([p, c], f16, tag=f"c_{nm}", name=f"c_{nm}")
            nc.sync.dma_start(out=ct[nm][:], in_=view(nm))
        # fp16 scalars -> f32 working copy; per-scalar column APs
        scal32 = cpool.tile([C, 9], f32, tag="c_scal32", name="c_scal32")
        nc.vector.tensor_copy(out=scal32[:], in_=ct["scal"][:])
        for j, nm in enumerate(("sq", "bq", "sk", "bk", "bv", "sp1", "bp1",
                                "sw1", "bw1")):
            ct[nm] = scal32[:, j:j + 1]

        # ---- synthesize Sel / Ww1s / WpW1s on device -----------------
        # Sel[p, j] = 1 iff j // 16 == p  (i.e. 0 <= j - 16p <= 15)
        sel = cpool.tile([C, 16 * C], f16, tag="c_sel", name="c_sel")
        nc.gpsimd.memset(sel[:], 1.0)
        nc.gpsimd.affine_select(out=sel[:], in_=sel[:], compare_op=OP.is_ge,
                                fill=0.0, base=0, pattern=[[1, 16 * C]],
                                channel_multiplier=-16)
        nc.gpsimd.affine_select(out=sel[:], in_=sel[:], compare_op=OP.is_gt,
                                fill=0.0, base=16, pattern=[[-1, 16 * C]],
                                channel_multiplier=16)
        # Ww1s block i holds Ww1 at cols i*C + 8i .. +8 (rest zero)
        ww1s = cpool.tile([C, 16 * C], f16, tag="c_ww1s", name="c_ww1s")
        wpw1s = cpool.tile([C, 16 * C], f16, tag="c_wpw1s", name="c_wpw1s")
        nc.gpsimd.memset(ww1s[:], 0.0)
        nc.gpsimd.memset(wpw1s[:], 0.0)
        for i in range(16):
            c0 = i * C + 8 * i
            nc.vector.tensor_copy(out=ww1s[:, c0:c0 + 8], in_=ct["Ww1"][:])
            nc.vector.tensor_copy(out=wpw1s[:, c0:c0 + 8], in_=ct["P2W1"][:])

        # ---- qT = relu(bn(Wq.T @ qfT)) fp16 [C, M_LOC]; nqT = -qT -----
        qT = cpool.tile([C, M_LOC], f16, tag="c_qT", name="c_qT")
        nqT = cpool.tile([C, M_LOC], f16, tag="c_nqT", name="c_nqT")
        for t in range(M_LOC // CHUNK):
            q_ps = ps["kp"].tile([C, CHUNK], f32, tag="kp_t", name="q_ps")
            nc.tensor.matmul(out=q_ps[:], lhsT=ct["Wq"][:],
                             rhs=ct["qfT"][:, t * CHUNK:(t + 1) * CHUNK],
                             start=True, stop=True)
            nc.scalar.activation(out=qT[:, t * CHUNK:(t + 1) * CHUNK],
                                 in_=q_ps[:], func=AF.Relu,
                                 bias=ct["bq"], scale=ct["sq"])
            nc.scalar.activation(out=nqT[:, t * CHUNK:(t + 1) * CHUNK],
                                 in_=qT[:, t * CHUNK:(t + 1) * CHUNK],
                                 func=AF.Identity, scale=-1.0)

        outT = opool.tile([C, M_LOC], f32)

        for g in range(N_GROUPS):
            fT = gpool.tile([C, GROUP], f16, tag="fT")
            nc.sync.dma_start(out=fT[:],
                              in_=view("featT")[:, g * GROUP:(g + 1) * GROUP])
            pT = gpool.tile([3, GROUP], f16, tag="pT")
            nc.sync.dma_start(out=pT[:],
                              in_=view("posT")[:, g * GROUP:(g + 1) * GROUP])
            valT = vpool.tile([C, GROUP], f32, tag="valp")
            stacked_ps = ps["stk"].tile([C, CHUNK], f32, tag="stk_t", name="stacked_ps")
            # -------- phase A: per chunk of 512 gathered rows ---------
            for i in range(16):
                ch = g * 16 + i              # global chunk id
                q0 = ch * 32                 # first query of chunk
                ctx = fT[:, i * CHUNK:(i + 1) * CHUNK]
                pos = pT[:, i * CHUNK:(i + 1) * CHUNK]
                # key = relu(bn(Wk.T @ ctx))
                k_ps = ps["kp"].tile([C, CHUNK], f32, tag="kp_t", name="k_ps")
                nc.tensor.matmul(out=k_ps[:], lhsT=ct["Wk"][:], rhs=ctx,
                                 start=True, stop=True)
                keyT = spool.tile([C, CHUNK], f16, tag="keyT")
                nc.scalar.activation(out=keyT[:], in_=k_ps[:], func=AF.Relu,
                                     bias=ct["bk"], scale=ct["sk"])
                # pebx = relu(bn(Wp1.T @ pos))
                pebx_ps = ps["px"].tile([C, CHUNK], f32, tag="px_t", name="pebx_ps")
                nc.tensor.matmul(out=pebx_ps[:], lhsT=ct["Wp1"][:], rhs=pos,
                                 start=True, stop=True)
                pebxT = spool.tile([C, CHUNK], f16, tag="pebxT")
                nc.scalar.activation(out=pebxT[:], in_=pebx_ps[:], func=AF.Relu,
                                     bias=ct["bp1"], scale=ct["sp1"])
                # val = Wv.T @ ctx + Wp2.T @ pebx (+ bv + bp2 via bias)
                v_ps = ps["vp"].tile([C, CHUNK], f32, tag="vp_t", name="v_ps")
                nc.tensor.matmul(out=v_ps[:], lhsT=ct["Wv"][:], rhs=ctx,
                                 start=True, stop=False)
                nc.tensor.matmul(out=v_ps[:], lhsT=ct["Wp2"][:], rhs=pebxT[:],
                                 start=False, stop=True)
                nc.scalar.activation(out=valT[:, i * CHUNK:(i + 1) * CHUNK],
                                     in_=v_ps[:], func=AF.Identity,
                                     bias=ct["bv"], scale=1.0)
                # w1 logits, stacked: Ww1.T @ (key - q + peb) with
                # peb folded via WpW1s = Wp2 @ Ww1s and -q via nqT
                q_rep = nqT[:, q0:q0 + 32].unsqueeze(2).to_broadcast([C, 32, K])
                nc.tensor.matmul(out=stacked_ps[:],
                                 lhsT=ww1s[:, i * C:(i + 1) * C],
                                 rhs=keyT[:], start=(i == 0), stop=False,
                                 skip_group_check=True)
                nc.tensor.matmul(out=stacked_ps[:],
                                 lhsT=wpw1s[:, i * C:(i + 1) * C],
                                 rhs=pebxT[:], start=False, stop=False,
                                 skip_group_check=True)
                nc.tensor.matmul(out=stacked_ps[:],
                                 lhsT=ww1s[:, i * C:(i + 1) * C],
                                 rhs=q_rep, start=False, stop=(i == 15),
                                 skip_group_check=True)
            # -------- group tail: bn/relu, mm2, softmax ---------------
            stk_bn = spool.tile([C, CHUNK], f16, tag="stkbn")
            nc.scalar.activation(out=stk_bn[:], in_=stacked_ps[:], func=AF.Relu,
                                 bias=ct["bw1"], scale=ct["sw1"])
            w2_ps = ps["px"].tile([C, CHUNK], f32, tag="px_t", name="w2_ps")
            nc.tensor.matmul(out=w2_ps[:], lhsT=ct["W2bd"][:], rhs=stk_bn[:],
                             start=True, stop=True)
            mx = spool.tile([C, 32], f32, tag="mx")
            nc.vector.tensor_reduce(
                out=mx[:], in_=w2_ps[:].rearrange("p (m k) -> p m k", k=K),
                axis=mybir.AxisListType.X, op=OP.max)
            sm = spool.tile([C, CHUNK], f32, tag="sm")
            nc.vector.tensor_tensor(
                out=sm[:].rearrange("p (m k) -> p m k", k=K),
                in0=w2_ps[:].rearrange("p (m k) -> p m k", k=K),
                in1=mx[:].unsqueeze(2).to_broadcast([C, 32, K]),
                op=OP.subtract)
            e_t = spool.tile([C, CHUNK], f32, tag="e")
            nc.scalar.activation(out=e_t[:], in_=sm[:], func=AF.Exp)
            s_t = spool.tile([C, 32], f32, tag="s")
            nc.vector.tensor_reduce(
                out=s_t[:], in_=e_t[:].rearrange("p (m k) -> p m k", k=K),
                axis=mybir.AxisListType.X, op=OP.add)
            rinv = spool.tile([C, 32], f32, tag="rinv")
            nc.vector.reciprocal(out=rinv[:], in_=s_t[:])
            wf32 = spool.tile([C, CHUNK], f32, tag="wf32")
            nc.vector.tensor_tensor(
                out=wf32[:].rearrange("p (m k) -> p m k", k=K),
                in0=e_t[:].rearrange("p (m k) -> p m k", k=K),
                in1=rinv[:].unsqueeze(2).to_broadcast([C, 32, K]),
                op=OP.mult)
            wfin = spool.tile([C, CHUNK], f16, tag="wfin")
            nc.scalar.activation(out=wfin[:], in_=wf32[:], func=AF.Identity)
            # -------- phase B: weighted sum per chunk -----------------
            for i in range(16):
                ch = g * 16 + i
                wrep_ps = ps["kp"].tile([C, CHUNK], f32, tag="kp_t", name="wrep_ps")
                nc.tensor.matmul(out=wrep_ps[:],
                                 lhsT=sel[:, i * C:(i + 1) * C],
                                 rhs=wfin[:], start=True, stop=True)
                prod = spool.tile([C, CHUNK], f32, tag="prod")
                nc.vector.tensor_tensor(out=prod[:],
                                        in0=valT[:, i * CHUNK:(i + 1) * CHUNK],
                                        in1=wrep_ps[:], op=OP.mult)
                nc.vector.tensor_reduce(
                    out=outT[:, ch * 32:(ch + 1) * 32],
                    in_=prod[:].rearrange("p (m k) -> p m k", k=K),
                    axis=mybir.AxisListType.X, op=OP.add)

        outT16 = opool.tile([C, M_LOC], f16, tag="out16", name="out16")
        nc.scalar.activation(out=outT16[:], in_=outT[:], func=AF.Identity)
        nc.sync.dma_start(out=out_d[:], in_=outT16[:])

    nc.compile()
    return nc


def _prep_inputs(inputs):
    """Host-side marshaling: per-core KNN gather, transpose, fp16 blob pack."""
    f = np.float32
    h = np.float16
    ctx_f = np.asarray(inputs["context_feat"], f)
    ctx_c = np.asarray(inputs["context_coord"], f)

    s = lambda g_: (np.asarray(g_, f) / np.sqrt(np.float32(1.0 + EPS_BN)))
    Wq = np.asarray(inputs["Wq"], f); Wk = np.asarray(inputs["Wk"], f)
    Wv = np.asarray(inputs["Wv"], f)
    Wp1 = np.asarray(inputs["Wp1"], f); Wp2 = np.asarray(inputs["Wp2"], f)
    Ww1 = np.asarray(inputs["Ww1"], f); Ww2 = np.asarray(inputs["Ww2"], f)

    sq = s(inputs["gq"]); bq = sq * inputs["bq"] + np.asarray(inputs["betaq"], f)
    sk = s(inputs["gk"]); bk = sk * inputs["bk"] + np.asarray(inputs["betak"], f)
    sp1 = s(inputs["gp1"])
    bp1 = sp1 * inputs["bp1"] + np.asarray(inputs["betap1"], f)
    bv = np.asarray(inputs["bv"], f) + np.asarray(inputs["bp2"], f)  # val bias
    # stacked bn for w1: row 8i+g ; fold bp2@Ww1 into bias
    sw1_g = s(inputs["gw1"])                                   # [G]
    bw1_g = (sw1_g * (np.asarray(inputs["bw1"], f)
                      + np.asarray(inputs["bp2"], f) @ Ww1)
             + np.asarray(inputs["betaw1"], f))                # [G]
    sw1 = np.tile(sw1_g, 16).astype(f)
    bw1 = np.tile(bw1_g, 16).astype(f)

    P2W1 = (Wp2 @ Ww1).astype(f)                               # [C, G]
    W2bd = np.zeros((C, C), f)
    for i in range(16):
        W2bd[8 * i:8 * i + 8, 8 * i:8 * i + 8] = Ww2

    scal = np.stack([sq, bq, sk, bk, bv, sp1, bp1, sw1, bw1], axis=1)  # [C,9]

    knn = np.asarray(inputs["knn_indexes"])
    knn = np.where(knn < 0, 0, knn).astype(np.int32)
    qf = np.asarray(inputs["query_feat"], f)
    qc = np.asarray(inputs["query_coord"], f)

    fixed = {"Wq": Wq, "Wk": Wk, "Wv": Wv, "Wp2": Wp2, "W2bd": W2bd,
             "Ww1": Ww1, "P2W1": P2W1, "scal": scal, "Wp1": Wp1}

    in_maps = []
    for c in range(N_CORES):
        sl = slice(c * M_LOC, (c + 1) * M_LOC)
        idx = knn[sl].reshape(-1)                        # [R_LOC] m*16+k order
        parts = dict(fixed)
        parts["featT"] = ctx_f[idx].T                    # [C, R_LOC]
        parts["posT"] = (ctx_c[idx] - np.repeat(qc[sl], K, axis=0)).T
        parts["qfT"] = qf[sl].T
        blob = np.empty(_NTOT, h)
        for nm, (p_, c_) in _LAYOUT:
            off = _OFFS[nm]
            blob[off:off + p_ * c_] = np.asarray(parts[nm], f).astype(h).ravel()
        in_maps.append({"blob": blob})
    return in_maps


def kernel(**inputs):
    global _compiled
    from concourse.bass_utils import run_bass_kernel_spmd
    if _compiled is None:
        _compiled = _build()
    in_maps = _prep_inputs(inputs)
    res = run_bass_kernel_spmd(_compiled, in_maps, core_ids=list(range(N_CORES)))
    out = np.concatenate([res.results[c]["out"].T for c in range(N_CORES)], axis=0)
    return np.ascontiguousarray(out.astype(np.float32))


# revision 5
# speedup vs baseline: 12.7512x; 1.4986x over previous
"""KNN grouped-vector-attention pool kernel for 8 Trainium2 NeuronCores.

Strategy: shard queries M=16384 across 8 cores (2048 each). The context
feature table is sharded across cores (16384 rows each, fp16) and
reassembled on device with an HBM AllGather; each core then resolves its own
KNN gathers locally via indirect DMA and XBAR DMA-transposes into
channel-major layout. Relative positions (tiny) are pre-gathered on the
host. All per-core inputs are packed into one contiguous fp16 blob (~5.2MB)
so the host->device path pays a single transfer per core (per-array fixed
cost dominates this interconnect). Structured matrices (Sel / Ww1s / WpW1s)
are synthesized on device from tiny seeds. All matmuls run fp16 x fp16 with
fp32 PSUM accumulation; output returns as fp16.
"""
import sys
sys.path.insert(0, '/opt/trn_rl_repo')
import numpy as np

N_CORES = 8
M, N, K, C, G = 16384, 131072, 16, 128, 8
M_LOC = M // N_CORES          # 2048 queries per core
R_LOC = M_LOC * K             # 32768 gathered rows per core
N_LOC = N // N_CORES          # 16384 context rows uploaded per core
CHUNK = 512                   # rows per compute chunk (one PSUM bank)
GROUP = 16 * CHUNK            # 8192 rows per stacked group
N_GROUPS = R_LOC // GROUP     # 4
EPS_BN = 1e-5

# blob layout: (name, (partitions, cols)) packed row-major, fp16
_LAYOUT = [
    ("ctxslice", (C, N_LOC)),    # this core's context-feat rows, flat
    ("qfT", (C, M_LOC)),
    ("Wq", (C, C)), ("Wk", (C, C)), ("Wv", (C, C)),
    ("Wp2", (C, C)), ("W2bd", (C, C)),
    ("Ww1", (C, G)), ("P2W1", (C, G)),
    ("scal", (C, 9)),            # sq,bq,sk,bk,bv,sp1,bp1,sw1,bw1
    ("knn16", (C, R_LOC // C * 2)),  # [128,256] i32 KNN blocks, fp16 bits
    ("posT", (3, R_LOC)),
    ("Wp1", (3, C)),
]
_OFFS = {}
_NTOT = 0
for _nm, (_p, _c) in _LAYOUT:
    _OFFS[_nm] = _NTOT
    _NTOT += _p * _c

_compiled = None


def _build():
    from concourse import bacc, bass, mybir
    import concourse.tile as tile

    f32 = mybir.dt.float32
    f16 = mybir.dt.float16
    i32 = mybir.dt.int32
    AF = mybir.ActivationFunctionType
    OP = mybir.AluOpType

    nc = bacc.Bacc("TRN2", target_bir_lowering=False, debug=False,
                   num_devices=N_CORES)

    blob = nc.dram_tensor("blob", (_NTOT,), f16, kind="ExternalInput").ap()
    out_d = nc.dram_tensor("out", (C, M_LOC), f16, kind="ExternalOutput").ap()

    def view(nm):
        p, c = dict(_LAYOUT)[nm]
        off = _OFFS[nm]
        return blob[off:off + p * c].rearrange("(p c) -> p c", p=p)

    from contextlib import ExitStack
    est = ExitStack()
    with tile.TileContext(nc) as tc, est:
        dpool = est.enter_context(tc.tile_pool(name="dram", bufs=1, space="DRAM"))
        cpool = est.enter_context(tc.tile_pool(name="const", bufs=1))
        gtpool = est.enter_context(tc.tile_pool(name="gt", bufs=4))
        gpool = est.enter_context(tc.tile_pool(name="gath", bufs=2))
        vpool = est.enter_context(tc.tile_pool(name="valp", bufs=2))
        spool = est.enter_context(tc.tile_pool(name="work", bufs=2))
        opool = est.enter_context(tc.tile_pool(name="outp", bufs=1))
        ps = {}
        for nm, nb in [("kp", 2), ("px", 2), ("vp", 2), ("stk", 1)]:
            ps[nm] = est.enter_context(tc.tile_pool(name=nm, bufs=nb, space="PSUM"))

        # ---- AllGather the context-feature table in HBM --------------
        ib = dpool.tile([C, N_LOC], f16, tag="ib", name="ib")
        ob = dpool.tile([C, N_LOC * N_CORES], f16, tag="ob", name="ob")
        nc.gpsimd.dma_start(ib[:], view("ctxslice"))
        nc.gpsimd.collective_compute(
            "AllGather", OP.bypass,
            replica_groups=[list(range(N_CORES))],
            ins=[ib.opt()], outs=[ob.opt()])
        # reinterpret the gathered flat buffer as [N, C] row-major
        ctx2d = ob[:].rearrange("p (r c) -> (p r) c", c=C)

        # ---- constants into SBUF -------------------------------------
        ct = {}
        for nm in ("qfT", "Wq", "Wk", "Wv", "Wp2", "W2bd", "Ww1", "P2W1",
                   "scal", "knn16", "Wp1"):
            p, c = dict(_LAYOUT)[nm]
            ct[nm] = cpool.tile([p, c], f16, tag=f"c_{nm}", name=f"c_{nm}")
            nc.sync.dma_start(out=ct[nm][:], in_=view(nm))
        knn32 = ct["knn16"][:].bitcast(i32)          # [128, R_LOC/128] i32
        # fp16 scalars -> f32 working copy; per-scalar column APs
        scal32 = cpool.tile([C, 9], f32, tag="c_scal32", name="c_scal32")
        nc.vector.tensor_copy(out=scal32[:], in_=ct["scal"][:])
        for j, nm in enumerate(("sq", "bq", "sk", "bk", "bv", "sp1", "bp1",
                                "sw1", "bw1")):
            ct[nm] = scal32[:, j:j + 1]

        # ---- synthesize Sel / Ww1s / WpW1s on device -----------------
        # Sel[p, j] = 1 iff j // 16 == p  (i.e. 0 <= j - 16p <= 15)
        sel = cpool.tile([C, 16 * C], f16, tag="c_sel", name="c_sel")
        nc.gpsimd.memset(sel[:], 1.0)
        nc.gpsimd.affine_select(out=sel[:], in_=sel[:], compare_op=OP.is_ge,
                                fill=0.0, base=0, pattern=[[1, 16 * C]],
                                channel_multiplier=-16)
        nc.gpsimd.affine_select(out=sel[:], in_=sel[:], compare_op=OP.is_gt,
                                fill=0.0, base=16, pattern=[[-1, 16 * C]],
                                channel_multiplier=16)
        # Ww1s block i holds Ww1 at cols i*C + 8i .. +8 (rest zero)
        ww1s = cpool.tile([C, 16 * C], f16, tag="c_ww1s", name="c_ww1s")
        wpw1s = cpool.tile([C, 16 * C], f16, tag="c_wpw1s", name="c_wpw1s")
        nc.gpsimd.memset(ww1s[:], 0.0)
        nc.gpsimd.memset(wpw1s[:], 0.0)
        for i in range(16):
            c0 = i * C + 8 * i
            nc.vector.tensor_copy(out=ww1s[:, c0:c0 + 8], in_=ct["Ww1"][:])
            nc.vector.tensor_copy(out=wpw1s[:, c0:c0 + 8], in_=ct["P2W1"][:])

        # ---- qT = relu(bn(Wq.T @ qfT)) fp16 [C, M_LOC]; nqT = -qT -----
        qT = cpool.tile([C, M_LOC], f16, tag="c_qT", name="c_qT")
        nqT = cpool.tile([C, M_LOC], f16, tag="c_nqT", name="c_nqT")
        for t in range(M_LOC // CHUNK):
            q_ps = ps["kp"].tile([C, CHUNK], f32, tag="kp_t", name="q_ps")
            nc.tensor.matmul(out=q_ps[:], lhsT=ct["Wq"][:],
                             rhs=ct["qfT"][:, t * CHUNK:(t + 1) * CHUNK],
                             start=True, stop=True)
            nc.scalar.activation(out=qT[:, t * CHUNK:(t + 1) * CHUNK],
                                 in_=q_ps[:], func=AF.Relu,
                                 bias=ct["bq"], scale=ct["sq"])
            nc.scalar.activation(out=nqT[:, t * CHUNK:(t + 1) * CHUNK],
                                 in_=qT[:, t * CHUNK:(t + 1) * CHUNK],
                                 func=AF.Identity, scale=-1.0)

        outT = opool.tile([C, M_LOC], f32)

        for g in range(N_GROUPS):
            fT = gpool.tile([C, GROUP], f16, tag="fT")
            # gather + transpose this group's 8192 neighbor rows
            for blk in range(GROUP // C):
                gcol = g * (GROUP // C) + blk
                gt = gtpool.tile([C, C], f16, tag="gt")
                nc.gpsimd.indirect_dma_start(
                    out=gt[:], out_offset=None,
                    in_=ctx2d,
                    in_offset=bass.IndirectOffsetOnAxis(
                        ap=knn32[:, gcol:gcol + 1], axis=0))
                nc.sync.dma_start_transpose(
                    out=fT[:, blk * C:(blk + 1) * C], in_=gt[:])
            pT = gpool.tile([3, GROUP], f16, tag="pT")
            nc.sync.dma_start(out=pT[:],
                              in_=view("posT")[:, g * GROUP:(g + 1) * GROUP])
            valT = vpool.tile([C, GROUP], f32, tag="valp")
            stacked_ps = ps["stk"].tile([C, CHUNK], f32, tag="stk_t", name="stacked_ps")
            # -------- phase A: per chunk of 512 gathered rows ---------
            for i in range(16):
                ch = g * 16 + i              # global chunk id
                q0 = ch * 32                 # first query of chunk
                ctx = fT[:, i * CHUNK:(i + 1) * CHUNK]
                pos = pT[:, i * CHUNK:(i + 1) * CHUNK]
                # key = relu(bn(Wk.T @ ctx))
                k_ps = ps["kp"].tile([C, CHUNK], f32, tag="kp_t", name="k_ps")
                nc.tensor.matmul(out=k_ps[:], lhsT=ct["Wk"][:], rhs=ctx,
                                 start=True, stop=True)
                keyT = spool.tile([C, CHUNK], f16, tag="keyT")
                nc.scalar.activation(out=keyT[:], in_=k_ps[:], func=AF.Relu,
                                     bias=ct["bk"], scale=ct["sk"])
                # pebx = relu(bn(Wp1.T @ pos))
                pebx_ps = ps["px"].tile([C, CHUNK], f32, tag="px_t", name="pebx_ps")
                nc.tensor.matmul(out=pebx_ps[:], lhsT=ct["Wp1"][:], rhs=pos,
                                 start=True, stop=True)
                pebxT = spool.tile([C, CHUNK], f16, tag="pebxT")
                nc.scalar.activation(out=pebxT[:], in_=pebx_ps[:], func=AF.Relu,
                                     bias=ct["bp1"], scale=ct["sp1"])
                # val = Wv.T @ ctx + Wp2.T @ pebx (+ bv + bp2 via bias)
                v_ps = ps["vp"].tile([C, CHUNK], f32, tag="vp_t", name="v_ps")
                nc.tensor.matmul(out=v_ps[:], lhsT=ct["Wv"][:], rhs=ctx,
                                 start=True, stop=False)
                nc.tensor.matmul(out=v_ps[:], lhsT=ct["Wp2"][:], rhs=pebxT[:],
                                 start=False, stop=True)
                nc.scalar.activation(out=valT[:, i * CHUNK:(i + 1) * CHUNK],
                                     in_=v_ps[:], func=AF.Identity,
                                     bias=ct["bv"], scale=1.0)
                # w1 logits, stacked: Ww1.T @ (key - q + peb) with
                # peb folded via WpW1s = Wp2 @ Ww1s and -q via nqT
                q_rep = nqT[:, q0:q0 + 32].unsqueeze(2).to_broadcast([C, 32, K])
                nc.tensor.matmul(out=stacked_ps[:],
                                 lhsT=ww1s[:, i * C:(i + 1) * C],
                                 rhs=keyT[:], start=(i == 0), stop=False,
                                 skip_group_check=True)
                nc.tensor.matmul(out=stacked_ps[:],
                                 lhsT=wpw1s[:, i * C:(i + 1) * C],
                                 rhs=pebxT[:], start=False, stop=False,
                                 skip_group_check=True)
                nc.tensor.matmul(out=stacked_ps[:],
                                 lhsT=ww1s[:, i * C:(i + 1) * C],
                                 rhs=q_rep, start=False, stop=(i == 15),
                                 skip_group_check=True)
            # -------- group tail: bn/relu, mm2, softmax ---------------
            stk_bn = spool.tile([C, CHUNK], f16, tag="stkbn")
            nc.scalar.activation(out=stk_bn[:], in_=stacked_ps[:], func=AF.Relu,
                                 bias=ct["bw1"], scale=ct["sw1"])
            w2_ps = ps["px"].tile([C, CHUNK], f32, tag="px_t", name="w2_ps")
            nc.tensor.matmul(out=w2_ps[:], lhsT=ct["W2bd"][:], rhs=stk_bn[:],
                             start=True, stop=True)
            mx = spool.tile([C, 32], f32, tag="mx")
            nc.vector.tensor_reduce(
                out=mx[:], in_=w2_ps[:].rearrange("p (m k) -> p m k", k=K),
                axis=mybir.AxisListType.X, op=OP.max)
            sm = spool.tile([C, CHUNK], f32, tag="sm")
            nc.vector.tensor_tensor(
                out=sm[:].rearrange("p (m k) -> p m k", k=K),
                in0=w2_ps[:].rearrange("p (m k) -> p m k", k=K),
                in1=mx[:].unsqueeze(2).to_broadcast([C, 32, K]),
                op=OP.subtract)
            e_t = spool.tile([C, CHUNK], f32, tag="e")
            nc.scalar.activation(out=e_t[:], in_=sm[:], func=AF.Exp)
            s_t = spool.tile([C, 32], f32, tag="s")
            nc.vector.tensor_reduce(
                out=s_t[:], in_=e_t[:].rearrange("p (m k) -> p m k", k=K),
                axis=mybir.AxisListType.X, op=OP.add)
            rinv = spool.tile([C, 32], f32, tag="rinv")
            nc.vector.reciprocal(out=rinv[:], in_=s_t[:])
            wf32 = spool.tile([C, CHUNK], f32, tag="wf32")
            nc.vector.tensor_tensor(
                out=wf32[:].rearrange("p (m k) -> p m k", k=K),
                in0=e_t[:].rearrange("p (m k) -> p m k", k=K),
                in1=rinv[:].unsqueeze(2).to_broadcast([C, 32, K]),
                op=OP.mult)
            wfin = spool.tile([C, CHUNK], f16, tag="wfin")
            nc.scalar.activation(out=wfin[:], in_=wf32[:], func=AF.Identity)
            # -------- phase B: weighted sum per chunk -----------------
            for i in range(16):
                ch = g * 16 + i
                wrep_ps = ps["kp"].tile([C, CHUNK], f32, tag="kp_t", name="wrep_ps")
                nc.tensor.matmul(out=wrep_ps[:],
                                 lhsT=sel[:, i * C:(i + 1) * C],
                                 rhs=wfin[:], start=True, stop=True)
                prod = spool.tile([C, CHUNK], f32, tag="prod")
                nc.vector.tensor_tensor(out=prod[:],
                                        in0=valT[:, i * CHUNK:(i + 1) * CHUNK],
                                        in1=wrep_ps[:], op=OP.mult)
                nc.vector.tensor_reduce(
                    out=outT[:, ch * 32:(ch + 1) * 32],
                    in_=prod[:].rearrange("p (m k) -> p m k", k=K),
                    axis=mybir.AxisListType.X, op=OP.add)

        outT16 = opool.tile([C, M_LOC], f16, tag="out16", name="out16")
        nc.scalar.activation(out=outT16[:], in_=outT[:], func=AF.Identity)
        nc.sync.dma_start(out=out_d[:], in_=outT16[:])

    nc.compile()
    return nc


def _prep_inputs(inputs):
    """Host-side marshaling: shard context, gather positions, fp16 blob pack."""
    f = np.float32
    h = np.float16
    ctx_f = np.asarray(inputs["context_feat"], f)
    ctx_c = np.asarray(inputs["context_coord"], f)
    ctx16 = ctx_f.astype(h)                                    # [N, C]

    s = lambda g_: (np.asarray(g_, f) / np.sqrt(np.float32(1.0 + EPS_BN)))
    Wq = np.asarray(inputs["Wq"], f); Wk = np.asarray(inputs["Wk"], f)
    Wv = np.asarray(inputs["Wv"], f)
    Wp1 = np.asarray(inputs["Wp1"], f); Wp2 = np.asarray(inputs["Wp2"], f)
    Ww1 = np.asarray(inputs["Ww1"], f); Ww2 = np.asarray(inputs["Ww2"], f)

    sq = s(inputs["gq"]); bq = sq * inputs["bq"] + np.asarray(inputs["betaq"], f)
    sk = s(inputs["gk"]); bk = sk * inputs["bk"] + np.asarray(inputs["betak"], f)
    sp1 = s(inputs["gp1"])
    bp1 = sp1 * inputs["bp1"] + np.asarray(inputs["betap1"], f)
    bv = np.asarray(inputs["bv"], f) + np.asarray(inputs["bp2"], f)  # val bias
    # stacked bn for w1: row 8i+g ; fold bp2@Ww1 into bias
    sw1_g = s(inputs["gw1"])                                   # [G]
    bw1_g = (sw1_g * (np.asarray(inputs["bw1"], f)
                      + np.asarray(inputs["bp2"], f) @ Ww1)
             + np.asarray(inputs["betaw1"], f))                # [G]
    sw1 = np.tile(sw1_g, 16).astype(f)
    bw1 = np.tile(bw1_g, 16).astype(f)

    P2W1 = (Wp2 @ Ww1).astype(f)                               # [C, G]
    W2bd = np.zeros((C, C), f)
    for i in range(16):
        W2bd[8 * i:8 * i + 8, 8 * i:8 * i + 8] = Ww2

    scal = np.stack([sq, bq, sk, bk, bv, sp1, bp1, sw1, bw1], axis=1)  # [C,9]

    knn = np.asarray(inputs["knn_indexes"])
    knn = np.where(knn < 0, 0, knn).astype(np.int32)
    qf = np.asarray(inputs["query_feat"], f)
    qc = np.asarray(inputs["query_coord"], f)

    fixed = {"Wq": Wq, "Wk": Wk, "Wv": Wv, "Wp2": Wp2, "W2bd": W2bd,
             "Ww1": Ww1, "P2W1": P2W1, "scal": scal, "Wp1": Wp1}
    fixed16 = {nm: np.asarray(v, f).astype(h).ravel() for nm, v in fixed.items()}

    in_maps = []
    for c in range(N_CORES):
        sl = slice(c * M_LOC, (c + 1) * M_LOC)
        idx = knn[sl].reshape(-1)                        # [R_LOC] m*16+k order
        knn_t = idx.reshape(R_LOC // C, C).T.copy()      # [128, R_LOC/128] i32
        blob = np.empty(_NTOT, h)
        pieces = dict(fixed16)
        pieces["ctxslice"] = ctx16[c * N_LOC:(c + 1) * N_LOC].ravel()
        pieces["qfT"] = qf[sl].T.astype(h).ravel()
        pieces["knn16"] = knn_t.view(h).ravel()
        pieces["posT"] = (ctx_c[idx] - np.repeat(qc[sl], K, axis=0)) \
            .T.astype(h).ravel()
        for nm, (p_, c_) in _LAYOUT:
            off = _OFFS[nm]
            blob[off:off + p_ * c_] = pieces[nm]
        in_maps.append({"blob": blob})
    return in_maps


def kernel(**inputs):
    global _compiled
    from concourse.bass_utils import run_bass_kernel_spmd
    if _compiled is None:
        _compiled = _build()
    in_maps = _prep_inputs(inputs)
    res = run_bass_kernel_spmd(_compiled, in_maps, core_ids=list(range(N_CORES)))
    out = np.concatenate([res.results[c]["out"].T for c in range(N_CORES)], axis=0)
    return np.ascontiguousarray(out.astype(np.float32))
